# revision 1
# baseline (speedup 1.0000x reference)
"""HINGCN (metapath GCN) Trainium2 kernel — 8-core SPMD, node-dim sharded.

Reference computation (N=8192, F=128, H=32, M=3 metapaths, C=16 classes):
    h1 = relu(A[m] @ (x @ W1[m]) + b1[m])          per metapath
    h2 = relu(A[m] @ (h1 @ W2[m]) + b2[m])
    e  = leaky_relu(h2 . a, 0.2); attn = softmax_m(e)
    out = sum_m attn[m] * h2[m];  logits = relu(out @ W_lin + b_lin)
    return log_softmax(logits)

Sharding: core k owns output rows u in [1024k, 1024k+1024); x/weights are
replicated, the [N,H*M] support matrix for layer 2 is AllGathered between the
layers (fp8 payload, 98KB/rank).

Perf design (from trace analysis of the bf16 baseline: PE 357us busy under a
50% power throttle, adjacency stream ~300GB/s on one HWDGE ring, ~100us dead
zone around the AllGather, ~70us serial attention tail):
  - adjacency is pre-scaled by SA=8192 and quantized host-side to fp8 e4m3;
    support matrices S1/S2 are quantized to fp8 on-device. GCN matmuls run in
    MatmulPerfMode.DoubleRow (K=256 per instruction) — 2x PE throughput and
    half the HBM bytes vs bf16. The 1/SA (and 1/(SA*SW) for layer 2, W2
    pre-scaled by SW=256 to keep S2 in fp8 range) is folded into the ReLU
    activation's input scale.
  - host lays A_k^T out partition-major ([m, p, vt*1024+u]) so each 1MB chunk
    DMA is 128 contiguous 8KB descriptors; chunks alternate between the SP and
    ACT HWDGE rings so fixed costs overlap.
  - metapath 0's whole A-block (8.4MB fp8) is cached in SBUF during layer 1;
    layer 2 processes m=0 first with zero DMA, filling the AllGather bubble
    while the rings prefetch m=1/m=2.
  - S2 staging, AllGather bounce and the gathered-S2 unpack all ride the
    gpsimd SWDGE queue, so the adjacency rings never stall on the collective.
  - the attention/head tail is matmul-broadcast based (no PE transposes): e
    rows via K=32 matmuls, attn broadcast to 32 partitions via K=1 matmuls,
    head via K=33 matmuls with a ones-row providing the bias path; softmax /
    log_softmax skip max-subtraction (logits are O(1e-2), exp is safe).
"""

import numpy as np
import ml_dtypes
from contextlib import ExitStack


def _ensure_ntff_hook_module():
    """concourse.bass_utils imports antenv.axon_hooks when tracing is
    requested (including via BASS_TRACE=1 in the environment); some images
    lack that submodule, which would raise ModuleNotFoundError mid-run.
    Install a functional stand-in when it's missing."""
    import sys
    import types
    try:
        import antenv.axon_hooks  # noqa: F401
        return
    except Exception:
        pass
    mod = types.ModuleType("antenv.axon_hooks")
    hook = [None]
    mod.set_axon_ntff_profile_hook = lambda h: hook.__setitem__(0, h)
    mod.get_axon_ntff_profile_hook = lambda: hook[0]
    try:
        import antenv
        antenv.axon_hooks = mod
    except Exception:
        pass
    sys.modules["antenv.axon_hooks"] = mod
    try:
        from trn_agent_boot.trn_boot import _ntff_profile_via_ctypes
        h = _ntff_profile_via_ctypes("/opt/axon/libaxon_pjrt.so")
        if h is not None:
            mod.set_axon_ntff_profile_hook(h)
    except Exception:
        pass


_ensure_ntff_hook_module()

import concourse.bass as bass
import concourse.tile as tile
from concourse import bacc, mybir
from concourse.bass_utils import run_bass_kernel_spmd

NCORES = 8
N, F, H, M, C = 8192, 128, 32, 3, 16
UL = N // NCORES          # rows per core (1024)
VT = N // 128             # v-tiles (64)
UT = UL // 128            # u-tiles per core (8)
CH = 4                    # v-tiles per DMA chunk (4KB/partition, 512KB total)
NCHUNK = VT // CH         # chunks per metapath (8)
ALPHA = 0.2
SA = 8192.0               # adjacency pre-scale (A*SA in [0,1) fits e4m3)
SW = 256.0                # W2 pre-scale so S2*SW ~ N(0,1) fits e4m3

BF = mybir.dt.bfloat16
F32 = mybir.dt.float32
FP8 = mybir.dt.float8e4
AX = mybir.AxisListType.X
AF = mybir.ActivationFunctionType
OP = mybir.AluOpType
DR = mybir.MatmulPerfMode.DoubleRow

NP_FP8 = ml_dtypes.float8_e4m3


def build_kernel_body(nc, tc, ctx, t_in, out_dram):
    xt, at, w1, w2, b1t, b2t, acol, wlin = (
        t_in["xt"], t_in["at"], t_in["w1"], t_in["w2"],
        t_in["b1t"], t_in["b2t"], t_in["acol"], t_in["wlin"])

    const = ctx.enter_context(tc.tile_pool(name="const", bufs=1))
    sbuf = ctx.enter_context(tc.tile_pool(name="sbuf", bufs=1))
    atp1 = ctx.enter_context(tc.tile_pool(name="atp1", bufs=4))
    atp2 = ctx.enter_context(tc.tile_pool(name="atp2", bufs=5))
    psum = ctx.enter_context(tc.tile_pool(name="psum", bufs=2, space="PSUM"))
    dram = ctx.enter_context(tc.tile_pool(name="dram", bufs=1, space="DRAM"))

    ring = [0]

    def dma_ring(dst, src):
        eng = nc.sync if ring[0] % 2 == 0 else nc.scalar
        ring[0] += 1
        eng.dma_start(dst, src)

    # ---- constants / parameters in SBUF ----
    w1_sb = const.tile([128, M * H], BF)
    nc.sync.dma_start(w1_sb[:], w1[:])
    w2_sb = const.tile([H, M * H], BF)
    nc.sync.dma_start(w2_sb[:], w2[:])
    b1t_sb = const.tile([H, M], F32)
    nc.scalar.dma_start(b1t_sb[:], b1t[:])
    b2t_sb = const.tile([H, M], F32)
    nc.scalar.dma_start(b2t_sb[:], b2t[:])
    acol_sb = const.tile([H, 1], BF)
    nc.scalar.dma_start(acol_sb[:], acol[:])
    wlin_sb = const.tile([H + 1, C], BF)
    nc.scalar.dma_start(wlin_sb[:], wlin[:])
    ones_col = const.tile([1, H], BF)
    nc.vector.memset(ones_col[:], 1.0)
    one_one = const.tile([1, 1], BF)
    nc.vector.memset(one_one[:], 1.0)

    xt_sb = const.tile([128, N], BF)
    for qq in range(4):
        dma_ring(xt_sb[:, qq * N // 4:(qq + 1) * N // 4],
                 xt[:, qq * N // 4:(qq + 1) * N // 4])

    at0_sb = const.tile([128, VT * UL], FP8)        # metapath-0 A cache
    at2_sb = const.tile([128, VT * UL // 4], FP8)   # metapath-2 quarter cache
    s1_sb = const.tile([128, VT * M * H], FP8)      # S1[v, (vt,m,h)]
    s2f_sb = [const.tile([128, VT * H], FP8, name=f"s2f_{m}")
              for m in range(M)]                    # gathered S2 per metapath
    h1t_sb = [const.tile([H, UL], BF, name=f"h1t_{m}") for m in range(M)]
    h2t_sb = [const.tile([H, UL], BF, name=f"h2t_{m}") for m in range(M)]
    s2stage = [const.tile([128, UT * H], FP8, name=f"s2stage{m}")
               for m in range(M)]                   # S2[m][(u%128), (ut,h)]
    out_t = const.tile([H + 1, UL], BF)             # attn-combined h2 + ones
    nc.vector.memset(out_t[H:H + 1, :], 1.0)
    lgr_all = const.tile([128, UT * C], F32)
    exp_all = const.tile([128, UT * C], F32)
    fin_all = const.tile([128, UT * C], F32)

    # ---- S1 = x @ W1 (all metapaths per matmul), cast to fp8 ----
    s1_4 = s1_sb[:].rearrange("p (m vt h) -> p m vt h", m=M, vt=VT)
    for vt in range(VT):
        ps1 = psum.tile([128, M * H], F32, tag="wide", name="ps1")
        nc.tensor.matmul(ps1[:], xt_sb[:, vt * 128:(vt + 1) * 128], w1_sb[:],
                         start=True, stop=True)
        nc.vector.tensor_copy(s1_4[:, :, vt, :],
                              ps1[:].rearrange("p (m h) -> p m h", m=M))

    # ---- GCN layer: ht[m] = relu(scale * sum_v A_q[m].T S_q + b) ----
    def gcn_layer(lhs3, bt_sb, ht_out, scale, fill_cache, atp, after_m=None):
        for m in range(M):
            acc = [psum.tile([H, 512], F32, tag="acc", bufs=4,
                             name=f"acc{m}_{s}") for s in range(2)]
            for c in range(NCHUNK):
                if m == 0:
                    attv = at0_sb[:, c * CH * UL:(c + 1) * CH * UL]
                    if fill_cache:
                        dma_ring(attv, at[0, :, c * CH * UL:(c + 1) * CH * UL])
                elif m == 2 and c < NCHUNK // 4:
                    attv = at2_sb[:, c * CH * UL:(c + 1) * CH * UL]
                    if fill_cache:
                        dma_ring(attv, at[2, :, c * CH * UL:(c + 1) * CH * UL])
                else:
                    a_t = atp.tile([128, CH * UL], FP8, tag="at", name="att")
                    dma_ring(a_t[:], at[m, :, c * CH * UL:(c + 1) * CH * UL])
                    attv = a_t[:]
                att3 = attv.rearrange("p (j u) -> p j u", j=CH)
                for j2 in range(CH // 2):
                    vt = c * CH + 2 * j2
                    lhs = lhs3(m)[:, vt:vt + 2, :]
                    st = (c == 0 and j2 == 0)
                    sp = (c == NCHUNK - 1 and j2 == CH // 2 - 1)
                    for s in range(2):
                        nc.tensor.matmul(
                            acc[s][:], lhs,
                            att3[:, 2 * j2:2 * j2 + 2, s * 512:(s + 1) * 512],
                            start=st, stop=sp, perf_mode=DR)
            for s in range(2):
                nc.scalar.activation(ht_out[m][:, s * 512:(s + 1) * 512],
                                     acc[s][:], AF.Relu,
                                     bias=bt_sb[:, m:m + 1], scale=scale)
            if after_m is not None:
                after_m(m)

    # S2[m] = h1[m] @ (W2[m]*SW) depends only on metapath m of layer 1:
    # exchange it as soon as that metapath finishes so all three AllGathers
    # overlap remaining layer-1 compute (the last one hides under layer-2 m=0,
    # which runs from the SBUF-cached adjacency with no DMA).
    def exchange_s2(m):
        for ut in range(UT):
            ps2 = psum.tile([128, H], F32, tag="wide", name="ps2")
            nc.tensor.matmul(ps2[:], h1t_sb[m][:, ut * 128:(ut + 1) * 128],
                             w2_sb[:, m * H:(m + 1) * H], start=True, stop=True)
            nc.scalar.copy(s2stage[m][:, ut * H:(ut + 1) * H], ps2[:])
        s2loc = dram.tile([128, UT * H], FP8, name=f"s2loc{m}")
        nc.gpsimd.dma_start(s2loc[:], s2stage[m][:])
        s2full = dram.tile([NCORES, 128, UT * H], FP8, addr_space="Shared",
                           name=f"s2full{m}")
        nc.gpsimd.collective_compute(
            "AllGather", OP.bypass,
            replica_groups=[list(range(NCORES))],
            ins=[s2loc[:].opt()], outs=[s2full[:].opt()])
        nc.gpsimd.dma_start(
            s2f_sb[m][:].rearrange("p (r c) -> p r c", r=NCORES),
            s2full[:].rearrange("r p c -> p r c"))

    s1_3 = [s1_4[:, m, :, :] for m in range(M)]
    gcn_layer(lambda m: s1_3[m], b1t_sb, h1t_sb, 1.0 / SA, fill_cache=True,
              atp=atp1, after_m=exchange_s2)

    s2f_3 = [s2f_sb[m][:].rearrange("p (vt h) -> p vt h", vt=VT)
             for m in range(M)]
    gcn_layer(lambda m: s2f_3[m], b2t_sb, h2t_sb, 1.0 / (SA * SW),
              fill_cache=False, atp=atp2)

    # ---- metapath attention (matmul-broadcast, no transposes) ----
    # e[m] rows [1, UL] via K=32 matmuls with `a` as the stationary column;
    # leaky-relu is fused into the PSUM->SBUF copy, exp skips max-subtraction
    # (|e| ~ 1e-2).
    exm, et = [], []
    for m in range(M):
        pe0 = psum.tile([1, 512], F32, tag="erow", name=f"pe{m}_0")
        pe1 = psum.tile([1, 512], F32, tag="erow", name=f"pe{m}_1")
        nc.tensor.matmul(pe0[:], acol_sb[:], h2t_sb[m][:, 0:512],
                         start=True, stop=True)
        nc.tensor.matmul(pe1[:], acol_sb[:], h2t_sb[m][:, 512:1024],
                         start=True, stop=True)
        etm = sbuf.tile([1, UL], BF, tag=f"et{m}", name=f"et{m}")
        nc.scalar.copy(etm[:, 0:512], pe0[:])
        nc.scalar.copy(etm[:, 512:1024], pe1[:])
        et.append(etm)
    for m in range(M):
        eta = sbuf.tile([1, UL], BF, tag="eta", name="eta")
        nc.vector.tensor_scalar_mul(eta[:], et[m][:], ALPHA)
        etl = sbuf.tile([1, UL], BF, tag="etl", name="etl")
        nc.vector.tensor_max(etl[:], et[m][:], eta[:])
        exr = sbuf.tile([1, UL], BF, tag=f"ex{m}", name=f"ex{m}")
        nc.scalar.activation(exr[:], etl[:], AF.Exp)
        exm.append(exr)
    ss01 = sbuf.tile([1, UL], F32, tag="ss01", name="ss01")
    nc.vector.tensor_add(ss01[:], exm[0][:], exm[1][:])
    ssum = sbuf.tile([1, UL], F32, tag="ssum", name="ssum")
    nc.vector.tensor_add(ssum[:], ss01[:], exm[2][:])
    ssum_bf = sbuf.tile([1, UL], BF, tag="ssum_bf", name="ssum_bf")
    nc.vector.tensor_copy(ssum_bf[:], ssum[:])

    # broadcast exp rows to 32 partitions via K=1 matmuls
    def bcast(row_bf, name):
        outt = sbuf.tile([H, UL], BF, tag=f"bc{name}", name=f"bc{name}")
        for s in range(2):
            pb = psum.tile([H, 512], F32, tag="acc", bufs=4,
                           name=f"pb{name}{s}")
            nc.tensor.matmul(pb[:], ones_col[:],
                             row_bf[:, s * 512:(s + 1) * 512],
                             start=True, stop=True)
            nc.scalar.copy(outt[:, s * 512:(s + 1) * 512], pb[:])
        return outt

    exb = [bcast(exm[m], f"e{m}") for m in range(M)]

    # out_t rows 0:H hold the UNNORMALIZED numerator sum_m exp_m * h2_m; row H
    # holds the softmax denominator. The head matmul then yields
    # num @ W_lin + den * b_lin, and scaling by 1/den inside the ReLU
    # activation gives relu(out @ W_lin + b_lin) exactly.
    t0 = sbuf.tile([H, UL], BF, tag="t0", name="t0")
    nc.vector.tensor_mul(t0[:], h2t_sb[0][:], exb[0][:])
    t1 = sbuf.tile([H, UL], BF, tag="t1", name="t1")
    nc.vector.tensor_mul(t1[:], h2t_sb[1][:], exb[1][:])
    t01 = sbuf.tile([H, UL], BF, tag="t01", name="t01")
    nc.vector.tensor_add(t01[:], t0[:], t1[:])
    t2 = sbuf.tile([H, UL], BF, tag="t2", name="t2")
    nc.vector.tensor_mul(t2[:], h2t_sb[2][:], exb[2][:])
    nc.vector.tensor_add(out_t[0:H, :], t01[:], t2[:])
    nc.vector.tensor_copy(out_t[H:H + 1, :], ssum_bf[:])

    # transpose the denominator row to u-partitions (8 tiny K=1 matmuls),
    # reciprocal on [128, 8] (8 elems/lane — fast)
    rsu = sbuf.tile([128, UT], F32, tag="rsu", name="rsu")
    for ut in range(UT):
        psr = psum.tile([128, 1], F32, tag="wide", name="psr")
        nc.tensor.matmul(psr[:], ssum_bf[0:1, ut * 128:(ut + 1) * 128],
                         one_one[:], start=True, stop=True)
        nc.scalar.copy(rsu[:, ut:ut + 1], psr[:])
    rsr = sbuf.tile([128, UT], F32, tag="rsr", name="rsr")
    nc.vector.reciprocal(rsr[:], rsu[:])

    # ---- linear head + log_softmax (batched over all u-tiles) ----
    for ut in range(UT):
        pslg = psum.tile([128, C], F32, tag="wide", name="pslg")
        nc.tensor.matmul(pslg[:], out_t[:, ut * 128:(ut + 1) * 128], wlin_sb[:],
                         start=True, stop=True)
        nc.scalar.activation(lgr_all[:, ut * C:(ut + 1) * C], pslg[:], AF.Relu,
                             scale=rsr[:, ut:ut + 1])
    nc.scalar.activation(exp_all[:], lgr_all[:], AF.Exp)  # logits >= 0, small
    sm = sbuf.tile([128, UT], F32, tag="sm", name="sm")
    nc.vector.reduce_sum(sm[:], exp_all[:].rearrange("p (u c) -> p u c", u=UT),
                         axis=AX)
    lsm = sbuf.tile([128, UT], F32, tag="lsm", name="lsm")
    nc.scalar.activation(lsm[:], sm[:], AF.Ln)
    for ut in range(UT):
        nc.vector.tensor_scalar_sub(fin_all[:, ut * C:(ut + 1) * C],
                                    lgr_all[:, ut * C:(ut + 1) * C],
                                    lsm[:, ut:ut + 1])
    nc.sync.dma_start(out_dram[:].rearrange("(ut p) c -> p ut c", p=128),
                      fin_all[:].rearrange("p (ut c) -> p ut c", ut=UT))


_CACHED = {}


def build():
    if "nc" in _CACHED:
        return _CACHED["nc"]
    nc = bacc.Bacc("TRN2", target_bir_lowering=False, debug=False,
                   num_devices=NCORES)
    t_in = {
        "xt": nc.dram_tensor("xt", [128, N], BF, kind="ExternalInput").ap(),
        "at": nc.dram_tensor("at", [M, 128, VT * UL], FP8,
                             kind="ExternalInput").ap(),
        "w1": nc.dram_tensor("w1", [128, M * H], BF, kind="ExternalInput").ap(),
        "w2": nc.dram_tensor("w2", [H, M * H], BF, kind="ExternalInput").ap(),
        "b1t": nc.dram_tensor("b1t", [H, M], F32, kind="ExternalInput").ap(),
        "b2t": nc.dram_tensor("b2t", [H, M], F32, kind="ExternalInput").ap(),
        "acol": nc.dram_tensor("acol", [H, 1], BF, kind="ExternalInput").ap(),
        "wlin": nc.dram_tensor("wlin", [H + 1, C], BF,
                               kind="ExternalInput").ap(),
    }
    out_dram = nc.dram_tensor("out", [UL, C], F32, kind="ExternalOutput").ap()
    with tile.TileContext(nc) as tc, ExitStack() as ctx:
        build_kernel_body(nc, tc, ctx, t_in, out_dram)
    nc.compile()
    _CACHED["nc"] = nc
    return nc


def _bf16(x):
    """Fast f32 -> bf16 with round-to-nearest-even via integer ops."""
    x = np.ascontiguousarray(x, dtype=np.float32)
    u = x.view(np.uint32)
    r = ((u + 0x7FFF + ((u >> 16) & 1)) >> 16).astype(np.uint16)
    return r.view(ml_dtypes.bfloat16)


def make_in_maps(x, adjs, W1, b1, W2, b2, a, W_lin, b_lin):
    xt = np.ascontiguousarray(_bf16(x).T)                       # [128, N]
    w1 = np.ascontiguousarray(_bf16(W1).transpose(1, 0, 2)).reshape(128, M * H)
    w2 = np.ascontiguousarray(
        _bf16(np.asarray(W2) * SW).transpose(1, 0, 2)).reshape(H, M * H)
    b1t = np.ascontiguousarray(b1.T, dtype=np.float32)          # [H, M]
    b2t = np.ascontiguousarray(b2.T, dtype=np.float32)
    acol = np.ascontiguousarray(_bf16(a).reshape(H, 1))
    wlin = _bf16(np.concatenate([np.asarray(W_lin),
                                 np.asarray(b_lin)[None, :]], axis=0))
    aq = (np.asarray(adjs, dtype=np.float32) * SA).astype(NP_FP8)  # [M, N, N]
    in_maps = []
    for k in range(NCORES):
        blk = aq[:, k * UL:(k + 1) * UL, :]                     # [M, UL, N]
        blk = blk.reshape(M, UL, VT, 128).transpose(0, 3, 2, 1)  # [M,128,VT,UL]
        atk = np.ascontiguousarray(blk).reshape(M, 128, VT * UL)
        in_maps.append({"xt": xt, "at": atk, "w1": w1, "w2": w2,
                        "b1t": b1t, "b2t": b2t, "acol": acol, "wlin": wlin})
    return in_maps


def kernel(x, adjs, W1, b1, W2, b2, a, W_lin, b_lin, _trace=False,
           _trace_all=False):
    nc = build()
    in_maps = make_in_maps(x, adjs, W1, b1, W2, b2, a, W_lin, b_lin)
    kw = {}
    if _trace_all:
        kw["trace_cores"] = list(range(NCORES))
    res = run_bass_kernel_spmd(nc, in_maps, core_ids=list(range(NCORES)),
                               trace=_trace or _trace_all, **kw)
    out = np.concatenate([res.results[k]["out"] for k in range(NCORES)], axis=0)
    if _trace or _trace_all:
        kernel.last_result = res
    return out

